# revision 2
# baseline (speedup 1.0000x reference)
"""Subject-routed batched matmul for Trainium2 (8 NeuronCores, SPMD data-parallel).

out[b, d, t] = sum_c x[b, c, t] * weights[subjects[b], c, d]

Strategy:
- Data-parallel over batch B=128 across 8 cores (16 batches each).
- Host-side: gather per-batch weights (weights[subjects], tiny), cast x and w
  to fp16. The tolerance gate is rel_err < 2e-2 and fp16-in/fp16-out measures
  ~5e-4, so single fp16 (2 B/elem) halves HBM traffic vs the fp32-grade hi/lo
  scheme (4 B/elem): 34 MiB/core total vs 68 MiB -> ~100 us roofline at the
  358 GB/s per-core HBM limit.
- Device: per batch, out[b] (256d, 2048t) = w[b].T @ x[b], tiled K=2x128
  (contraction over c), M=2x128 (d -> PSUM partitions), N=4x512 (t, one
  PSUM bank per tile). 2 matmuls per PSUM bank (2 k-chunks), fp16 at
  1 cycle/row. Output copied PSUM f32 -> SBUF fp16 (DVE casts), stored fp16,
  upcast to fp32 on host.
- DMA: x loads on the SP HWDGE ring (nc.sync), stores on the ACT ring
  (nc.scalar), weights once via GPSIMD SWDGE. x loads are 1 MiB/batch with
  4 KiB contiguous per partition; stores 512 KiB with 4 KiB per partition.
"""

import sys

for _p in ("/opt/trn_rl_repo", "/root/.axon_site/_ro/trn_rl_repo"):
    if _p not in sys.path:
        sys.path.append(_p)

import numpy as np

import concourse.mybir as mybir
import concourse.tile as tile
from concourse import bacc
from concourse.bass_utils import run_bass_kernel_spmd

B, C, D, T, N_SUBJECTS = 128, 256, 256, 2048, 8
N_CORES = 8
BPC = B // N_CORES  # batches per core

KC = C // 128  # k chunks (contraction dim on partitions)
MC = D // 128  # m chunks (output partition dim)
NT = 512       # n tile (one PSUM bank of f32)
NC_ = T // NT  # n chunks

F32 = mybir.dt.float32
F16 = mybir.dt.float16

_compiled = None


def _build():
    nc = bacc.Bacc("TRN2", target_bir_lowering=False, debug=False)
    # x1[b, c, t] fp16 (plain reshape of the fp32 input, cast on host)
    # wp[p, b, k, d] fp16 — host-pre-packed to the SBUF layout so the weight
    # DMA is one fully contiguous 16 KiB/partition transfer.
    x_d = nc.dram_tensor("x1", [BPC, C, T], F16, kind="ExternalInput")
    w_d = nc.dram_tensor("wp", [128, BPC, KC, D], F16, kind="ExternalInput")
    o_d = nc.dram_tensor("out", [BPC, D, T], F16, kind="ExternalOutput")

    with tile.TileContext(nc) as tc:
        with (
            tc.tile_pool(name="wpool", bufs=1) as wpool,
            tc.tile_pool(name="xpool", bufs=6) as xpool,
            tc.tile_pool(name="opool", bufs=4) as opool,
            tc.tile_pool(name="psum", bufs=8, space="PSUM") as psum,
        ):
            # Weights resident for the whole kernel (2 MiB, contiguous per
            # partition). b=0's slice loads separately so the first matmuls
            # start fast; both ride the GPSIMD SWDGE path, which competes
            # with neither the x loads (SP ring) nor the stores (ACT ring).
            wt0 = wpool.tile([128, 1, KC, D], F16)
            wtr = wpool.tile([128, BPC - 1, KC, D], F16)
            # PE warmup: the HAM clock gate boots at 1.2 GHz and needs
            # ~3.4 us of sustained matmul activity to reach 2.4 GHz. Fill
            # the preamble-to-first-tile window with zero matmuls so the
            # real stream starts at full clock.
            warm = wpool.tile([128, 256], F16, name="warm")
            nc.gpsimd.memset(warm[:], 0.0)
            warmps = psum.tile([128, 256], F32, name="warmps", tag="pt")
            for _ in range(16):
                nc.tensor.matmul(
                    warmps[:], warm[:, :128], warm[:], start=True, stop=True
                )
            nc.gpsimd.dma_start(wt0[:], w_d[:, 0:1])
            nc.gpsimd.dma_start(wtr[:], w_d[:, 1:])

            for b in range(BPC):
                wt = wt0 if b == 0 else wtr
                wb = 0 if b == 0 else b - 1
                # xt[p, k, t] (1 MiB). For b=0 load in 4 t-chunks of 256 KiB
                # so the first matmuls start earlier; steady state uses one
                # 1 MiB DMA.
                xt = xpool.tile([128, KC, T], F16, tag="xt")
                xsrc = x_d[b].rearrange("(k p) t -> p k t", p=128)
                if b == 0:
                    for tch in range(NC_):
                        nc.sync.dma_start(
                            xt[:, :, tch * NT:(tch + 1) * NT],
                            xsrc[:, :, tch * NT:(tch + 1) * NT],
                        )
                else:
                    nc.sync.dma_start(xt[:], xsrc)
                for m in range(MC):
                    # ot[p, t] fp16 (512 KiB, stored as soon as this m is done)
                    ot = opool.tile([128, T], F16, tag="ot")
                    for n in range(NC_):
                        pt = psum.tile([128, NT], F32)
                        for k in range(KC):
                            nc.tensor.matmul(
                                pt[:],
                                wt[:, wb, k, m * 128:(m + 1) * 128],
                                xt[:, k, n * NT:(n + 1) * NT],
                                start=(k == 0),
                                stop=(k == KC - 1),
                            )
                        nc.vector.tensor_copy(ot[:, n * NT:(n + 1) * NT], pt[:])
                        if b == BPC - 1:
                            # tail: store each n-chunk as soon as it's copied
                            nc.scalar.dma_start(
                                o_d[b, m * 128:(m + 1) * 128, n * NT:(n + 1) * NT],
                                ot[:, n * NT:(n + 1) * NT],
                            )
                    if b < BPC - 1:
                        nc.scalar.dma_start(
                            o_d[b, m * 128:(m + 1) * 128, :], ot[:]
                        )

    nc.compile()
    return nc


def _get_compiled():
    global _compiled
    if _compiled is None:
        _compiled = _build()
    return _compiled


def _run(x, subjects, weights, **spmd_kwargs):
    x = np.asarray(x, dtype=np.float32)
    subjects = np.asarray(subjects).astype(np.int64)
    weights = np.asarray(weights, dtype=np.float32)

    x1 = x.astype(np.float16)              # (B, C, T) fp16
    w_g = weights[subjects].astype(np.float16)  # (B, C, D) fp16
    # wp[core][p, b, k, d] = w_g[core*BPC + b, k*128 + p, d]
    wp = np.ascontiguousarray(
        w_g.reshape(N_CORES, BPC, KC, 128, D).transpose(0, 3, 1, 2, 4)
    )

    nc = _get_compiled()
    in_maps = [
        {
            "x1": x1[i * BPC:(i + 1) * BPC],
            "wp": wp[i],
        }
        for i in range(N_CORES)
    ]
    res = run_bass_kernel_spmd(
        nc, in_maps, core_ids=list(range(N_CORES)), **spmd_kwargs
    )
    out = np.concatenate([r["out"] for r in res.results], axis=0).astype(
        np.float32
    )
    return out, res


def kernel(x, subjects, weights):
    return _run(x, subjects, weights)[0]
